# revision 11
# baseline (speedup 1.0000x reference)
"""3-layer GAT on 8 Trainium2 NeuronCores (Bass/Tile).

Sharding: nodes by contiguous range (6250/core); edges by dst range, sorted
by dst. Per layer: dense phase computes [feat|el|er] = h @ [W|W.al|W.ar] for
local nodes -> AllGather the node table -> edge phase gathers table[src] rows
(dma_gather, int16 indices, 32768-row table split), builds one-hot(dst)
matrices on DVE, broadcasts er via a dst-indexed gather from the LOCAL table,
computes exp(leaky_relu(el+er)) on ACT, and aggregates (weighted feature sum +
softmax denominator) in one f32r matmul chain per 128-dst-node window into
PSUM. Epilogue normalizes, adds residual/bias, applies ELU (or the head-mean
for the output layer).

Softmax is computed without the segment-max subtraction: attention logits are
O(1) here so exp() cannot overflow, and the result is mathematically
identical.
"""
import sys

sys.path.insert(0, "/opt/trn_rl_repo")

import numpy as np

# ---- problem constants (nn_GAT_3951369912452) ----
N = 50000
E = 800000
IN = 256
HID = 64
H = 4
C = 40
SLOPE = 0.2
NCORES = 8
NLOC = N // NCORES          # 6250
P = 128
W = (NLOC + P - 1) // P     # 49 windows/core
SPLIT = 32768               # int16 gather index limit

F0 = H * HID                # 256 feat width, layers 0/1
F2 = H * C                  # 160 feat width, layer 2
ROW0 = 320                  # table row floats, layers 0/1 (1280B, 256B-aligned)
ROW2 = 192                  # table row floats, layer 2 (768B)
EL0, ER0 = 256, 260         # el/er col offsets in table rows, layers 0/1
EL2, ER2 = 160, 164         # layer 2
RHS0 = 260                  # matmul rhs cols (feat + ex), layers 0/1
RHS2 = 164                  # layer 2

_CACHE = {}


# ======================= host preprocessing =======================

def _fold_w(Wm, al, ar):
    Hh, D = al.shape
    Wal = np.stack([Wm[:, h * D:(h + 1) * D] @ al[h] for h in range(Hh)], axis=1)
    War = np.stack([Wm[:, h * D:(h + 1) * D] @ ar[h] for h in range(Hh)], axis=1)
    return Wal.astype(np.float32), War.astype(np.float32)


def _wrap16(block):
    """int16 idx list (cap,) -> dma_gather wrapped layout (128, cap//16)."""
    cap = block.shape[0]
    wb = block.reshape(cap // 16, 16).T
    return np.tile(wb, (8, 1)).astype(np.int16)


def _preprocess(inputs):
    x = np.asarray(inputs["x"], np.float32)
    src = np.asarray(inputs["src"], np.int64)
    dst = np.asarray(inputs["dst"], np.int64)

    Wcat = []
    for l, (Wm, al, ar) in enumerate(
        [(inputs["W0"], inputs["al0"], inputs["ar0"]),
         (inputs["W1"], inputs["al1"], inputs["ar1"]),
         (inputs["W2"], inputs["al2"], inputs["ar2"])]
    ):
        Wm = np.asarray(Wm, np.float32)
        Wal, War = _fold_w(Wm, np.asarray(al, np.float32), np.asarray(ar, np.float32))
        parts = [Wm, Wal, War]
        if l == 2:
            # residual projection, pre-scaled by the head-mean 1/H factor
            parts.append(np.asarray(inputs["Wres2"], np.float32) / H)
        Wcat.append(np.ascontiguousarray(np.concatenate(parts, axis=1)))

    biases = [np.asarray(inputs[k], np.float32).reshape(-1) for k in ("b0", "b1", "b2")]
    use_bias = [bool(np.any(b != 0)) for b in biases]
    brep = [np.tile(b.reshape(1, -1), (P, 1)).astype(np.float32) for b in biases]
    brep[2] = brep[2] / H  # folded head-mean

    order = np.argsort(dst, kind="stable")
    ds = dst[order]
    ss = src[order]

    # per (core, window) edge lists
    per_core = []
    KA = KB = 1
    for r in range(NCORES):
        lo = r * NLOC
        m = (ds >= lo) & (ds < lo + NLOC)
        ld = ds[m] - lo
        ls = ss[m]
        wins = []
        for w in range(W):
            wm = (ld >= w * P) & (ld < (w + 1) * P)
            dw = ld[wm] - w * P
            sw = ls[wm]
            a = sw < SPLIT
            sa, da = sw[a], dw[a]
            sb, db = sw[~a] - SPLIT, dw[~a]
            wins.append((sa, da, sb, db))
            KA = max(KA, -(-max(len(sa), 1) // P))
            KB = max(KB, -(-max(len(sb), 1) // P))
        per_core.append(wins)
    KT = KA + KB

    in_maps = []
    for r in range(NCORES):
        idxA = np.full((W, KA * P), -1, np.int16)
        idxB = np.full((W, KB * P), -1, np.int16)
        idxE = np.zeros((W, KT * P), np.int16)
        drel = np.full((W, KT * P), -1.0, np.float32)
        meta = np.zeros((2 * W,), np.int32)
        for w, (sa, da, sb, db) in enumerate(per_core[r]):
            na, nb = len(sa), len(sb)
            idxA[w, :na] = sa.astype(np.int16)
            idxB[w, :nb] = sb.astype(np.int16)
            if na == 0:
                idxA[w, 0] = 0
            if nb == 0:
                idxB[w, 0] = 0
            meta[w] = max(na, 1)
            meta[W + w] = max(nb, 1)
            drel[w, :na] = da.astype(np.float32)
            drel[w, KA * P:KA * P + nb] = db.astype(np.float32)
            idxE[w, :na] = (da + w * P).astype(np.int16)
            idxE[w, KA * P:KA * P + nb] = (db + w * P).astype(np.int16)

        in_maps.append({
            "x": np.ascontiguousarray(x[r * NLOC:(r + 1) * NLOC]),
            "Wcat0": Wcat[0], "Wcat1": Wcat[1], "Wcat2": Wcat[2],
            "idxA": np.hstack([_wrap16(idxA[w]) for w in range(W)]),
            "idxB": np.hstack([_wrap16(idxB[w]) for w in range(W)]),
            "idxE": np.hstack([_wrap16(idxE[w]) for w in range(W)]),
            "drel": np.hstack([drel[w].reshape(KT, P).T for w in range(W)]).astype(np.float32),
            "meta": meta.reshape(1, 2 * W),
            **({"brep0": brep[0]} if use_bias[0] else {}),
            **({"brep1": brep[1]} if use_bias[1] else {}),
            **({"brep2": brep[2]} if use_bias[2] else {}),
        })

    meta_prog = {"KA": KA, "KB": KB, "use_bias": tuple(use_bias)}
    return in_maps, meta_prog


# ======================= device program =======================

def _build(meta_prog):
    import concourse.bass as bass
    import concourse.bacc as bacc
    import concourse.mybir as mybir
    import concourse.tile as tile
    from concourse.masks import make_identity

    KA, KB = meta_prog["KA"], meta_prog["KB"]
    use_bias = meta_prog["use_bias"]
    KT = KA + KB
    f32 = mybir.dt.float32
    f32r = mybir.dt.float32r
    bf16 = mybir.dt.bfloat16
    i16 = mybir.dt.int16
    i32 = mybir.dt.int32
    AF = mybir.ActivationFunctionType
    OP = mybir.AluOpType

    nc = bacc.Bacc("TRN2", target_bir_lowering=False, debug=False,
                   num_devices=NCORES)

    # ---- I/O ----
    x_d = nc.dram_tensor("x", [NLOC, F0], f32, kind="ExternalInput")
    Wc_d = [nc.dram_tensor(f"Wcat{l}", [IN, cols], f32r, kind="ExternalInput")
            for l, cols in enumerate((F0 + 8, F0 + 8, F2 + 8 + F2))]
    idxA_d = nc.dram_tensor("idxA", [P, W * KA * 8], i16, kind="ExternalInput")
    idxB_d = nc.dram_tensor("idxB", [P, W * KB * 8], i16, kind="ExternalInput")
    idxE_d = nc.dram_tensor("idxE", [P, W * KT * 8], i16, kind="ExternalInput")
    drel_d = nc.dram_tensor("drel", [P, W * KT], f32, kind="ExternalInput")
    meta_d = nc.dram_tensor("meta", [1, 2 * W], i32, kind="ExternalInput")
    brep_d = [
        nc.dram_tensor(f"brep{l}", [P, (F0, F0, F2)[l]], f32, kind="ExternalInput")
        if use_bias[l] else None
        for l in range(3)
    ]
    out_d = nc.dram_tensor("out", [NLOC, C], f32, kind="ExternalOutput")

    # ---- internal DRAM ----
    tinf = [nc.dram_tensor(f"tin{l}", [(NLOC + 1) * row], f32r)
            for l, row in enumerate((ROW0, ROW0, ROW2))]
    tab = [nc.dram_tensor(f"tab{l}", [N, row], f32r, addr_space="Shared")
           for l, row in enumerate((ROW0, ROW0, ROW2))]
    h_d = [x_d,
           nc.dram_tensor("h1", [NLOC, F0], f32),
           nc.dram_tensor("h2", [NLOC, F0], f32)]
    res2_d = nc.dram_tensor("res2", [NLOC, F2], f32)

    ROWS = (ROW0, ROW0, ROW2)
    ELS = (EL0, EL0, EL2)
    ERS = (ER0, ER0, ER2)
    FS = (F0, F0, F2)
    RHSS = (RHS0, RHS0, RHS2)
    # dense cols copied into the table rows; for L2 the row is filled to its
    # full 192 width with real psum data (res2 head) so gathered rows carry no
    # uninitialized bytes
    DCOLS = (F0 + 8, F0 + 8, ROW2)

    with tile.TileContext(nc) as tc:
        with (
            tc.tile_pool(name="const", bufs=1) as cp,
            tc.tile_pool(name="work", bufs=2) as wp,
            tc.tile_pool(name="psum", bufs=2, space="PSUM") as pp,
        ):
            # ---- persistent tiles ----
            iota_t = cp.tile([P, P], f32)
            nc.gpsimd.iota(iota_t[:], pattern=[[1, P]], base=0,
                           channel_multiplier=0,
                           allow_small_or_imprecise_dtypes=True)
            ident_t = cp.tile([P, P], f32)
            make_identity(nc, ident_t[:])

            idxA_t = cp.tile([P, W * KA * 8], i16)
            nc.sync.dma_start(idxA_t[:], idxA_d[:, :])
            idxB_t = cp.tile([P, W * KB * 8], i16)
            nc.sync.dma_start(idxB_t[:], idxB_d[:, :])
            idxE_t = cp.tile([P, W * KT * 8], i16)
            nc.sync.dma_start(idxE_t[:], idxE_d[:, :])
            drel_t = cp.tile([P, W * KT], f32)
            nc.sync.dma_start(drel_t[:], drel_d[:, :])
            meta_t = cp.tile([1, 2 * W], i32)
            nc.sync.dma_start(meta_t[:], meta_d[:, :])

            Wc_t = []
            for l in range(3):
                cols = (F0 + 8, F0 + 8, F2 + 8 + F2)[l]
                chunks = []
                for k in range(2):
                    t = cp.tile([P, cols], f32r, tag=f"wc{l}{k}")
                    nc.sync.dma_start(t[:], Wc_d[l][k * P:(k + 1) * P, :])
                    chunks.append(t)
                Wc_t.append(chunks)
            brep_t = []
            for l in range(3):
                if use_bias[l]:
                    t = cp.tile([P, (F0, F0, F2)[l]], f32, tag=f"brep{l}")
                    nc.sync.dma_start(t[:], brep_d[l][:, :])
                    brep_t.append(t)
                else:
                    brep_t.append(None)

            # gather destinations: manual double buffer, zeroed once so pad
            # lanes (not overwritten by the exact-count gathers) stay finite
            G_bufs = []
            for i in range(2):
                g = cp.tile([P, KT * ROW0], f32r, tag=f"G{i}")
                nc.vector.memset(g[:, :].bitcast(f32), 0.0)
                G_bufs.append(g)

            regs = [nc.gpsimd.alloc_register(f"gr{i}") for i in range(6)]

            # table row views (flat -> rows; er view starts at the er column)
            tin_rows = [tinf[l][0:NLOC * ROWS[l]].rearrange("(n c) -> n c", c=ROWS[l])
                        for l in range(3)]
            er_view = [tinf[l][ERS[l]:ERS[l] + NLOC * ROWS[l]]
                       .rearrange("(n c) -> n c", c=ROWS[l])[:, 0:64]
                       for l in range(3)]

            def dense_phase(l):
                cols = (F0 + 8, F0 + 8, F2 + 8 + F2)[l]
                for w in range(W):
                    nw = min(P, NLOC - w * P)
                    ht = wp.tile([P, F0], f32, tag="ht")
                    nc.sync.dma_start(ht[0:nw, :], h_d[l][w * P:w * P + nw, :])
                    psd = pp.tile([P, cols], f32, tag="psd")
                    for k in range(2):
                        pst = pp.tile([P, P], f32, tag="pst")
                        nc.tensor.transpose(out=pst[:], in_=ht[:, k * P:(k + 1) * P],
                                            identity=ident_t[:])
                        hT = wp.tile([P, P], f32r, tag="hT")
                        nc.vector.tensor_copy(out=hT[:], in_=pst[:])
                        nc.tensor.matmul(out=psd[:], lhsT=hT[:], rhs=Wc_t[l][k][:],
                                         start=(k == 0), stop=(k == 1))
                    # write full-width table rows so every gathered byte is a
                    # finite float (pad cols zeroed for L0/L1)
                    do = wp.tile([P, ROWS[l]], f32r, tag="do")
                    nc.vector.tensor_copy(out=do[:, 0:DCOLS[l]],
                                          in_=psd[:, 0:DCOLS[l]])
                    if DCOLS[l] < ROWS[l]:
                        nc.vector.memset(do[:, DCOLS[l]:ROWS[l]].bitcast(f32), 0.0)
                    nc.sync.dma_start(tin_rows[l][w * P:w * P + nw, :],
                                      do[0:nw, :])
                    if l == 2:
                        r2 = wp.tile([P, F2], f32, tag="r2")
                        nc.vector.tensor_copy(out=r2[:], in_=psd[:, F2 + 8:F2 + 8 + F2])
                        nc.sync.dma_start(res2_d[w * P:w * P + nw, :], r2[0:nw, :])

            import os
            dbg_taps = []

            def _tap(name, ap, shape):
                d = nc.dram_tensor(name, shape, ap.dtype, kind="ExternalOutput")
                nc.sync.dma_start(d[tuple(slice(0, s) for s in shape)], ap)
                dbg_taps.append(name)

            def edge_phase(l):
                ROW, EL, F, RHS = ROWS[l], ELS[l], FS[l], RHSS[l]
                D = F // H
                for w in range(W):
                    nw = min(P, NLOC - w * P)
                    G = G_bufs[w % 2][:, 0:KT * ROW].rearrange(
                        "p (t c) -> p t c", c=ROW)
                    rA = regs[(2 * w) % 6]
                    rB = regs[(2 * w + 1) % 6]
                    nc.gpsimd.reg_load(rA, meta_t[0:1, w:w + 1])
                    nc.gpsimd.dma_gather(
                        out_ap=G[:, 0:KA, :], in_ap=tab[l][0:SPLIT, :],
                        idxs_ap=idxA_t[:, w * KA * 8:(w + 1) * KA * 8],
                        num_idxs=KA * P, num_idxs_reg=rA, elem_size=ROW,
                        single_packet=False)
                    nc.gpsimd.reg_load(rB, meta_t[0:1, W + w:W + w + 1])
                    nc.gpsimd.dma_gather(
                        out_ap=G[:, KA:KT, :], in_ap=tab[l][SPLIT:N, :],
                        idxs_ap=idxB_t[:, w * KB * 8:(w + 1) * KB * 8],
                        num_idxs=KB * P, num_idxs_reg=rB, elem_size=ROW,
                        single_packet=False)
                    ert = wp.tile([P, KT, 64], f32r, tag="ert")
                    nc.gpsimd.dma_gather(
                        out_ap=ert[:, :, :], in_ap=er_view[l],
                        idxs_ap=idxE_t[:, w * KT * 8:(w + 1) * KT * 8],
                        num_idxs=KT * P, num_idxs_reg=KT * P, elem_size=64,
                        elem_step=ROW, single_packet=False)

                    ohdt = f32r if l < 2 else bf16
                    oh = wp.tile([P, KT, P], ohdt, tag="oh")
                    nc.vector.tensor_tensor(
                        out=oh[:, :, :],
                        in0=iota_t[:, None, :].to_broadcast([P, KT, P]),
                        in1=drel_t[:, w * KT:(w + 1) * KT, None].to_broadcast([P, KT, P]),
                        op=OP.is_equal)

                    ext = wp.tile([P, KT, 4], f32, tag="ext")
                    nc.vector.tensor_add(ext[:, :, :],
                                         ert[:, :, 0:4].bitcast(f32),
                                         G[:, :, EL:EL + 4].bitcast(f32))
                    nc.vector.scalar_tensor_tensor(
                        out=ext[:, :, :], in0=ext[:, :, :], scalar=SLOPE,
                        in1=ext[:, :, :], op0=OP.mult, op1=OP.max)
                    nc.scalar.activation(ext[:, :, :], ext[:, :, :], AF.Exp)

                    rdt = f32r if l < 2 else bf16
                    rhs = wp.tile([P, KT, RHS], rdt, tag="rhs")
                    nc.vector.tensor_tensor(
                        out=rhs[:, :, 0:F].rearrange("p t (h d) -> p t h d", h=H),
                        in0=G[:, :, 0:F].bitcast(f32).rearrange(
                            "p t (h d) -> p t h d", h=H),
                        in1=ext[:, :, :, None].to_broadcast([P, KT, H, D]),
                        op=OP.mult)
                    nc.vector.tensor_copy(out=rhs[:, :, F:F + 4], in_=ext[:, :, :])

                    psw = pp.tile([P, RHS], f32, tag="psw")
                    for t in range(KT):
                        nc.tensor.matmul(out=psw[:, :], lhsT=oh[:, t, :],
                                         rhs=rhs[:, t, :],
                                         start=(t == 0), stop=(t == KT - 1))

                    if os.environ.get("GAT_DBG") and l == int(os.environ.get("GAT_DBG_L", "0")) and w == 0:
                        _tap("d_G", G[:, :, :].bitcast(f32), [P, KT, ROW])
                        _tap("d_ert", ert[:, :, :].bitcast(f32), [P, KT, 64])
                        _tap("d_ext", ext[:, :, :], [P, KT, 4])
                        _tap("d_oh", oh[:, :, :].bitcast(f32) if l < 2 else oh[:, :, :],
                             [P, KT, P])
                        _tap("d_rhs", rhs[:, :, :] if l == 2 else rhs[:, :, :].bitcast(f32), [P, KT, RHS])
                        psb = wp.tile([P, RHS], f32, tag="d_psb")
                        nc.vector.tensor_copy(out=psb[:], in_=psw[:, :])
                        _tap("d_psw", psb[:], [P, RHS])

                    dn = wp.tile([P, 4], f32, tag="dn")
                    if l < 2:
                        nc.vector.tensor_scalar_max(dn[:], psw[:, F:F + 4], 1e-30)
                    else:
                        nc.vector.tensor_scalar(dn[:], psw[:, F:F + 4],
                                                1e-30, float(H), OP.max, OP.mult)
                    rec = wp.tile([P, 4], f32, tag="rec")
                    nc.vector.reciprocal(rec[:], dn[:])

                    of = wp.tile([P, F], f32, tag="of")
                    nc.vector.tensor_tensor(
                        out=of[:].rearrange("p (h d) -> p h d", h=H),
                        in0=psw[:, 0:F].rearrange("p (h d) -> p h d", h=H),
                        in1=rec[:, :, None].to_broadcast([P, H, D]),
                        op=OP.mult)
                    if l == 1:
                        rt = wp.tile([P, F0], f32, tag="rt")
                        nc.sync.dma_start(rt[0:nw, :], h_d[1][w * P:w * P + nw, :])
                        nc.vector.tensor_add(of[:], of[:], rt[:])
                    elif l == 2:
                        rt = wp.tile([P, F2], f32, tag="rt")
                        nc.sync.dma_start(rt[0:nw, :], res2_d[w * P:w * P + nw, :])
                        nc.vector.tensor_add(of[:], of[:], rt[:])
                    if use_bias[l]:
                        nc.vector.tensor_add(of[:], of[:], brep_t[l][:])

                    if l < 2:
                        # ELU: out = (x - 1 - min(x,0)) + exp(min(x,0))
                        t0 = wp.tile([P, F0], f32, tag="t0")
                        nc.vector.tensor_scalar_min(t0[:], of[:], 0.0)
                        oh_out = wp.tile([P, F0], f32, tag="oh_out")
                        nc.vector.scalar_tensor_tensor(
                            out=oh_out[:], in0=of[:], scalar=-1.0, in1=t0[:],
                            op0=OP.add, op1=OP.subtract)
                        nc.scalar.activation(t0[:], t0[:], AF.Exp)
                        nc.vector.tensor_add(oh_out[:], oh_out[:], t0[:])
                        nc.sync.dma_start(h_d[l + 1][w * P:w * P + nw, :],
                                          oh_out[0:nw, :])
                    else:
                        msum = wp.tile([P, C], f32, tag="msum")
                        nc.vector.tensor_reduce(
                            msum[:],
                            of[:].rearrange("p (h c) -> p c h", h=H),
                            axis=mybir.AxisListType.X, op=OP.add)
                        nc.sync.dma_start(out_d[w * P:w * P + nw, :], msum[0:nw, :])

            for l in range(3):
                dense_phase(l)
                nc.gpsimd.collective_compute(
                    "AllGather", OP.bypass,
                    replica_groups=[list(range(NCORES))],
                    ins=[tin_rows[l][:, :].opt()],
                    outs=[tab[l][:, :].opt()])
                edge_phase(l)

    nc.compile()
    return nc


# ======================= entry point =======================

def kernel(**inputs) -> np.ndarray:
    from concourse.bass_utils import run_bass_kernel_spmd

    in_maps, meta_prog = _preprocess(inputs)
    key = (meta_prog["KA"], meta_prog["KB"], meta_prog["use_bias"])
    if key not in _CACHE:
        _CACHE[key] = _build(meta_prog)
    nc = _CACHE[key]
    res = run_bass_kernel_spmd(nc, in_maps, core_ids=list(range(NCORES)))
    return np.concatenate([r["out"] for r in res.results], axis=0)


# revision 12
# speedup vs baseline: 14.8421x; 14.8421x over previous
"""3-layer GAT on 8 Trainium2 NeuronCores (Bass/Tile).

Sharding: nodes by contiguous range (6250/core); edges by dst range, sorted
by dst. Per layer: dense phase computes [feat|el|er] = h @ [W|W.al|W.ar] for
local nodes -> AllGather the node table -> edge phase gathers table[src] rows
(dma_gather, int16 indices, 32768-row table split), builds one-hot(dst)
matrices on DVE, broadcasts er via a dst-indexed gather from the LOCAL table,
computes exp(leaky_relu(el+er)) on ACT, and aggregates (weighted feature sum +
softmax denominator) in one f32r matmul chain per 128-dst-node window into
PSUM. Epilogue normalizes, adds residual/bias, applies ELU (or the head-mean
for the output layer).

Softmax is computed without the segment-max subtraction: attention logits are
O(1) here so exp() cannot overflow, and the result is mathematically
identical.
"""
import sys

sys.path.insert(0, "/opt/trn_rl_repo")

import numpy as np

# ---- problem constants (nn_GAT_3951369912452) ----
N = 50000
E = 800000
IN = 256
HID = 64
H = 4
C = 40
SLOPE = 0.2
NCORES = 8
NLOC = N // NCORES          # 6250
P = 128
W = (NLOC + P - 1) // P     # 49 windows/core
SPLIT = 32768               # int16 gather index limit

F0 = H * HID                # 256 feat width, layers 0/1
F2 = H * C                  # 160 feat width, layer 2
ROW0 = 320                  # table row floats, layers 0/1 (1280B, 256B-aligned)
ROW2 = 192                  # table row floats, layer 2 (768B)
EL0, ER0 = 256, 260         # el/er col offsets in table rows, layers 0/1
EL2, ER2 = 160, 164         # layer 2
RHS0 = 260                  # matmul rhs cols (feat + ex), layers 0/1
RHS2 = 164                  # layer 2

_CACHE = {}


# ======================= host preprocessing =======================

def _fold_w(Wm, al, ar):
    Hh, D = al.shape
    Wal = np.stack([Wm[:, h * D:(h + 1) * D] @ al[h] for h in range(Hh)], axis=1)
    War = np.stack([Wm[:, h * D:(h + 1) * D] @ ar[h] for h in range(Hh)], axis=1)
    return Wal.astype(np.float32), War.astype(np.float32)


def _wrap16(block):
    """int16 idx list (cap,) -> dma_gather wrapped layout (128, cap//16)."""
    cap = block.shape[0]
    wb = block.reshape(cap // 16, 16).T
    return np.tile(wb, (8, 1)).astype(np.int16)


def _preprocess(inputs):
    x = np.asarray(inputs["x"], np.float32)
    src = np.asarray(inputs["src"], np.int64)
    dst = np.asarray(inputs["dst"], np.int64)

    Wcat = []
    for l, (Wm, al, ar) in enumerate(
        [(inputs["W0"], inputs["al0"], inputs["ar0"]),
         (inputs["W1"], inputs["al1"], inputs["ar1"]),
         (inputs["W2"], inputs["al2"], inputs["ar2"])]
    ):
        Wm = np.asarray(Wm, np.float32)
        Wal, War = _fold_w(Wm, np.asarray(al, np.float32), np.asarray(ar, np.float32))
        parts = [Wm, Wal, War]
        if l == 2:
            # residual projection, pre-scaled by the head-mean 1/H factor
            parts.append(np.asarray(inputs["Wres2"], np.float32) / H)
        Wcat.append(np.ascontiguousarray(np.concatenate(parts, axis=1)))

    biases = [np.asarray(inputs[k], np.float32).reshape(-1) for k in ("b0", "b1", "b2")]
    use_bias = [bool(np.any(b != 0)) for b in biases]
    brep = [np.tile(b.reshape(1, -1), (P, 1)).astype(np.float32) for b in biases]
    brep[2] = brep[2] / H  # folded head-mean

    order = np.argsort(dst, kind="stable")
    ds = dst[order]
    ss = src[order]

    # per (core, window) edge lists
    per_core = []
    KA = KB = 1
    for r in range(NCORES):
        lo = r * NLOC
        m = (ds >= lo) & (ds < lo + NLOC)
        ld = ds[m] - lo
        ls = ss[m]
        wins = []
        for w in range(W):
            wm = (ld >= w * P) & (ld < (w + 1) * P)
            dw = ld[wm] - w * P
            sw = ls[wm]
            a = sw < SPLIT
            sa, da = sw[a], dw[a]
            sb, db = sw[~a] - SPLIT, dw[~a]
            wins.append((sa, da, sb, db))
            KA = max(KA, -(-max(len(sa), 1) // P))
            KB = max(KB, -(-max(len(sb), 1) // P))
        per_core.append(wins)
    KT = KA + KB

    in_maps = []
    for r in range(NCORES):
        idxA = np.full((W, KA * P), -1, np.int16)
        idxB = np.full((W, KB * P), -1, np.int16)
        idxE = np.zeros((W, KT * P), np.int16)
        drel = np.full((W, KT * P), -1.0, np.float32)
        meta = np.zeros((2 * W,), np.int32)
        for w, (sa, da, sb, db) in enumerate(per_core[r]):
            na, nb = len(sa), len(sb)
            idxA[w, :na] = sa.astype(np.int16)
            idxB[w, :nb] = sb.astype(np.int16)
            if na == 0:
                idxA[w, 0] = 0
            if nb == 0:
                idxB[w, 0] = 0
            meta[w] = max(na, 1)
            meta[W + w] = max(nb, 1)
            drel[w, :na] = da.astype(np.float32)
            drel[w, KA * P:KA * P + nb] = db.astype(np.float32)
            idxE[w, :na] = (da + w * P).astype(np.int16)
            idxE[w, KA * P:KA * P + nb] = (db + w * P).astype(np.int16)

        in_maps.append({
            "x": np.ascontiguousarray(x[r * NLOC:(r + 1) * NLOC]),
            "Wcat0": Wcat[0], "Wcat1": Wcat[1], "Wcat2": Wcat[2],
            "idxA": np.hstack([_wrap16(idxA[w]) for w in range(W)]),
            "idxB": np.hstack([_wrap16(idxB[w]) for w in range(W)]),
            "idxE": np.hstack([_wrap16(idxE[w]) for w in range(W)]),
            "drel": np.hstack([drel[w].reshape(KT, P).T for w in range(W)]).astype(np.float32),
            "meta": meta.reshape(1, 2 * W),
            **({"brep0": brep[0]} if use_bias[0] else {}),
            **({"brep1": brep[1]} if use_bias[1] else {}),
            **({"brep2": brep[2]} if use_bias[2] else {}),
        })

    meta_prog = {"KA": KA, "KB": KB, "use_bias": tuple(use_bias)}
    return in_maps, meta_prog


# ======================= device program =======================

def _build(meta_prog, repeat=1):
    import concourse.bass as bass
    import concourse.bacc as bacc
    import concourse.mybir as mybir
    import concourse.tile as tile
    from concourse.masks import make_identity

    KA, KB = meta_prog["KA"], meta_prog["KB"]
    use_bias = meta_prog["use_bias"]
    KT = KA + KB
    f32 = mybir.dt.float32
    f32r = mybir.dt.float32r
    bf16 = mybir.dt.bfloat16
    i16 = mybir.dt.int16
    i32 = mybir.dt.int32
    AF = mybir.ActivationFunctionType
    OP = mybir.AluOpType

    nc = bacc.Bacc("TRN2", target_bir_lowering=False, debug=False,
                   num_devices=NCORES)

    # ---- I/O ----
    x_d = nc.dram_tensor("x", [NLOC, F0], f32, kind="ExternalInput")
    Wc_d = [nc.dram_tensor(f"Wcat{l}", [IN, cols], f32r, kind="ExternalInput")
            for l, cols in enumerate((F0 + 8, F0 + 8, F2 + 8 + F2))]
    idxA_d = nc.dram_tensor("idxA", [P, W * KA * 8], i16, kind="ExternalInput")
    idxB_d = nc.dram_tensor("idxB", [P, W * KB * 8], i16, kind="ExternalInput")
    idxE_d = nc.dram_tensor("idxE", [P, W * KT * 8], i16, kind="ExternalInput")
    drel_d = nc.dram_tensor("drel", [P, W * KT], f32, kind="ExternalInput")
    meta_d = nc.dram_tensor("meta", [1, 2 * W], i32, kind="ExternalInput")
    brep_d = [
        nc.dram_tensor(f"brep{l}", [P, (F0, F0, F2)[l]], f32, kind="ExternalInput")
        if use_bias[l] else None
        for l in range(3)
    ]
    out_d = nc.dram_tensor("out", [NLOC, C], f32, kind="ExternalOutput")

    # ---- internal DRAM ----
    tinf = [nc.dram_tensor(f"tin{l}", [(NLOC + 1) * row], f32r)
            for l, row in enumerate((ROW0, ROW0, ROW2))]
    tab = [nc.dram_tensor(f"tab{l}", [N, row], f32r, addr_space="Shared")
           for l, row in enumerate((ROW0, ROW0, ROW2))]
    h_d = [x_d,
           nc.dram_tensor("h1", [NLOC, F0], f32),
           nc.dram_tensor("h2", [NLOC, F0], f32)]
    res2_d = nc.dram_tensor("res2", [NLOC, F2], f32)

    ROWS = (ROW0, ROW0, ROW2)
    ELS = (EL0, EL0, EL2)
    ERS = (ER0, ER0, ER2)
    FS = (F0, F0, F2)
    RHSS = (RHS0, RHS0, RHS2)
    # dense cols copied into the table rows; for L2 the row is filled to its
    # full 192 width with real psum data (res2 head) so gathered rows carry no
    # uninitialized bytes
    DCOLS = (F0 + 8, F0 + 8, ROW2)

    with tile.TileContext(nc) as tc:
        with (
            tc.tile_pool(name="const", bufs=1) as cp,
            tc.tile_pool(name="work", bufs=2) as wp,
            tc.tile_pool(name="psum", bufs=2, space="PSUM") as pp,
        ):
            # ---- persistent tiles ----
            iota_t = cp.tile([P, P], f32)
            nc.gpsimd.iota(iota_t[:], pattern=[[1, P]], base=0,
                           channel_multiplier=0,
                           allow_small_or_imprecise_dtypes=True)
            ident_t = cp.tile([P, P], f32)
            make_identity(nc, ident_t[:])

            idxA_t = cp.tile([P, W * KA * 8], i16)
            nc.sync.dma_start(idxA_t[:], idxA_d[:, :])
            idxB_t = cp.tile([P, W * KB * 8], i16)
            nc.sync.dma_start(idxB_t[:], idxB_d[:, :])
            idxE_t = cp.tile([P, W * KT * 8], i16)
            nc.sync.dma_start(idxE_t[:], idxE_d[:, :])
            drel_t = cp.tile([P, W * KT], f32)
            nc.sync.dma_start(drel_t[:], drel_d[:, :])
            meta_t = cp.tile([1, 2 * W], i32)
            nc.sync.dma_start(meta_t[:], meta_d[:, :])

            Wc_t = []
            for l in range(3):
                cols = (F0 + 8, F0 + 8, F2 + 8 + F2)[l]
                chunks = []
                for k in range(2):
                    t = cp.tile([P, cols], f32r, tag=f"wc{l}{k}")
                    nc.sync.dma_start(t[:], Wc_d[l][k * P:(k + 1) * P, :])
                    chunks.append(t)
                Wc_t.append(chunks)
            brep_t = []
            for l in range(3):
                if use_bias[l]:
                    t = cp.tile([P, (F0, F0, F2)[l]], f32, tag=f"brep{l}")
                    nc.sync.dma_start(t[:], brep_d[l][:, :])
                    brep_t.append(t)
                else:
                    brep_t.append(None)

            # gather destinations: manual double buffer, zeroed once so pad
            # lanes (not overwritten by the exact-count gathers) stay finite
            G_bufs = []
            for i in range(2):
                g = cp.tile([P, KT * ROW0], f32r, tag=f"G{i}")
                nc.vector.memset(g[:, :].bitcast(f32), 0.0)
                G_bufs.append(g)

            regs = [nc.gpsimd.alloc_register(f"gr{i}") for i in range(6)]

            # table row views (flat -> rows; er view starts at the er column)
            tin_rows = [tinf[l][0:NLOC * ROWS[l]].rearrange("(n c) -> n c", c=ROWS[l])
                        for l in range(3)]
            er_view = [tinf[l][ERS[l]:ERS[l] + NLOC * ROWS[l]]
                       .rearrange("(n c) -> n c", c=ROWS[l])[:, 0:64]
                       for l in range(3)]

            def dense_phase(l):
                cols = (F0 + 8, F0 + 8, F2 + 8 + F2)[l]
                for w in range(W):
                    nw = min(P, NLOC - w * P)
                    ht = wp.tile([P, F0], f32, tag="ht")
                    nc.sync.dma_start(ht[0:nw, :], h_d[l][w * P:w * P + nw, :])
                    psd = pp.tile([P, cols], f32, tag="psd")
                    for k in range(2):
                        pst = pp.tile([P, P], f32, tag="pst")
                        nc.tensor.transpose(out=pst[:], in_=ht[:, k * P:(k + 1) * P],
                                            identity=ident_t[:])
                        hT = wp.tile([P, P], f32r, tag="hT")
                        nc.vector.tensor_copy(out=hT[:], in_=pst[:])
                        nc.tensor.matmul(out=psd[:], lhsT=hT[:], rhs=Wc_t[l][k][:],
                                         start=(k == 0), stop=(k == 1))
                    # write full-width table rows so every gathered byte is a
                    # finite float (pad cols zeroed for L0/L1)
                    do = wp.tile([P, ROWS[l]], f32r, tag="do")
                    nc.vector.tensor_copy(out=do[:, 0:DCOLS[l]],
                                          in_=psd[:, 0:DCOLS[l]])
                    if DCOLS[l] < ROWS[l]:
                        nc.vector.memset(do[:, DCOLS[l]:ROWS[l]].bitcast(f32), 0.0)
                    nc.sync.dma_start(tin_rows[l][w * P:w * P + nw, :],
                                      do[0:nw, :])
                    if l == 2:
                        r2 = wp.tile([P, F2], f32, tag="r2")
                        nc.vector.tensor_copy(out=r2[:], in_=psd[:, F2 + 8:F2 + 8 + F2])
                        nc.sync.dma_start(res2_d[w * P:w * P + nw, :], r2[0:nw, :])

            import os
            dbg_taps = []

            def _tap(name, ap, shape):
                d = nc.dram_tensor(name, shape, ap.dtype, kind="ExternalOutput")
                nc.sync.dma_start(d[tuple(slice(0, s) for s in shape)], ap)
                dbg_taps.append(name)

            def edge_phase(l):
                ROW, EL, F, RHS = ROWS[l], ELS[l], FS[l], RHSS[l]
                D = F // H
                for w in range(W):
                    nw = min(P, NLOC - w * P)
                    G = G_bufs[w % 2][:, 0:KT * ROW].rearrange(
                        "p (t c) -> p t c", c=ROW)
                    rA = regs[(2 * w) % 6]
                    rB = regs[(2 * w + 1) % 6]
                    nc.gpsimd.reg_load(rA, meta_t[0:1, w:w + 1])
                    nc.gpsimd.dma_gather(
                        out_ap=G[:, 0:KA, :], in_ap=tab[l][0:SPLIT, :],
                        idxs_ap=idxA_t[:, w * KA * 8:(w + 1) * KA * 8],
                        num_idxs=KA * P, num_idxs_reg=rA, elem_size=ROW,
                        single_packet=False)
                    nc.gpsimd.reg_load(rB, meta_t[0:1, W + w:W + w + 1])
                    nc.gpsimd.dma_gather(
                        out_ap=G[:, KA:KT, :], in_ap=tab[l][SPLIT:N, :],
                        idxs_ap=idxB_t[:, w * KB * 8:(w + 1) * KB * 8],
                        num_idxs=KB * P, num_idxs_reg=rB, elem_size=ROW,
                        single_packet=False)
                    ert = wp.tile([P, KT, 64], f32r, tag="ert")
                    nc.gpsimd.dma_gather(
                        out_ap=ert[:, :, :], in_ap=er_view[l],
                        idxs_ap=idxE_t[:, w * KT * 8:(w + 1) * KT * 8],
                        num_idxs=KT * P, num_idxs_reg=KT * P, elem_size=64,
                        elem_step=ROW, single_packet=False)

                    ohdt = f32r if l < 2 else bf16
                    oh = wp.tile([P, KT, P], ohdt, tag="oh")
                    nc.vector.tensor_tensor(
                        out=oh[:, :, :],
                        in0=iota_t[:, None, :].to_broadcast([P, KT, P]),
                        in1=drel_t[:, w * KT:(w + 1) * KT, None].to_broadcast([P, KT, P]),
                        op=OP.is_equal)

                    ext = wp.tile([P, KT, 4], f32, tag="ext")
                    nc.vector.tensor_add(ext[:, :, :],
                                         ert[:, :, 0:4].bitcast(f32),
                                         G[:, :, EL:EL + 4].bitcast(f32))
                    nc.vector.scalar_tensor_tensor(
                        out=ext[:, :, :], in0=ext[:, :, :], scalar=SLOPE,
                        in1=ext[:, :, :], op0=OP.mult, op1=OP.max)
                    nc.scalar.activation(ext[:, :, :], ext[:, :, :], AF.Exp)

                    rdt = f32r if l < 2 else bf16
                    rhs = wp.tile([P, KT, RHS], rdt, tag="rhs")
                    nc.vector.tensor_tensor(
                        out=rhs[:, :, 0:F].rearrange("p t (h d) -> p t h d", h=H),
                        in0=G[:, :, 0:F].bitcast(f32).rearrange(
                            "p t (h d) -> p t h d", h=H),
                        in1=ext[:, :, :, None].to_broadcast([P, KT, H, D]),
                        op=OP.mult)
                    nc.vector.tensor_copy(out=rhs[:, :, F:F + 4], in_=ext[:, :, :])

                    psw = pp.tile([P, RHS], f32, tag="psw")
                    for t in range(KT):
                        nc.tensor.matmul(out=psw[:, :], lhsT=oh[:, t, :],
                                         rhs=rhs[:, t, :],
                                         start=(t == 0), stop=(t == KT - 1))

                    if os.environ.get("GAT_DBG") and l == int(os.environ.get("GAT_DBG_L", "0")) and w == 0:
                        _tap("d_G", G[:, :, :].bitcast(f32), [P, KT, ROW])
                        _tap("d_ert", ert[:, :, :].bitcast(f32), [P, KT, 64])
                        _tap("d_ext", ext[:, :, :], [P, KT, 4])
                        _tap("d_oh", oh[:, :, :].bitcast(f32) if l < 2 else oh[:, :, :],
                             [P, KT, P])
                        _tap("d_rhs", rhs[:, :, :] if l == 2 else rhs[:, :, :].bitcast(f32), [P, KT, RHS])
                        psb = wp.tile([P, RHS], f32, tag="d_psb")
                        nc.vector.tensor_copy(out=psb[:], in_=psw[:, :])
                        _tap("d_psw", psb[:], [P, RHS])

                    dn = wp.tile([P, 4], f32, tag="dn")
                    if l < 2:
                        nc.vector.tensor_scalar_max(dn[:], psw[:, F:F + 4], 1e-30)
                    else:
                        nc.vector.tensor_scalar(dn[:], psw[:, F:F + 4],
                                                1e-30, float(H), OP.max, OP.mult)
                    rec = wp.tile([P, 4], f32, tag="rec")
                    nc.vector.reciprocal(rec[:], dn[:])

                    of = wp.tile([P, F], f32, tag="of")
                    nc.vector.tensor_tensor(
                        out=of[:].rearrange("p (h d) -> p h d", h=H),
                        in0=psw[:, 0:F].rearrange("p (h d) -> p h d", h=H),
                        in1=rec[:, :, None].to_broadcast([P, H, D]),
                        op=OP.mult)
                    if l == 1:
                        rt = wp.tile([P, F0], f32, tag="rt")
                        nc.sync.dma_start(rt[0:nw, :], h_d[1][w * P:w * P + nw, :])
                        nc.vector.tensor_add(of[:], of[:], rt[:])
                    elif l == 2:
                        rt = wp.tile([P, F2], f32, tag="rt")
                        nc.sync.dma_start(rt[0:nw, :], res2_d[w * P:w * P + nw, :])
                        nc.vector.tensor_add(of[:], of[:], rt[:])
                    if use_bias[l]:
                        nc.vector.tensor_add(of[:], of[:], brep_t[l][:])

                    if l < 2:
                        # ELU: out = (x - 1 - min(x,0)) + exp(min(x,0))
                        t0 = wp.tile([P, F0], f32, tag="t0")
                        nc.vector.tensor_scalar_min(t0[:], of[:], 0.0)
                        oh_out = wp.tile([P, F0], f32, tag="oh_out")
                        nc.vector.scalar_tensor_tensor(
                            out=oh_out[:], in0=of[:], scalar=-1.0, in1=t0[:],
                            op0=OP.add, op1=OP.subtract)
                        nc.scalar.activation(t0[:], t0[:], AF.Exp)
                        nc.vector.tensor_add(oh_out[:], oh_out[:], t0[:])
                        nc.sync.dma_start(h_d[l + 1][w * P:w * P + nw, :],
                                          oh_out[0:nw, :])
                    else:
                        msum = wp.tile([P, C], f32, tag="msum")
                        nc.vector.tensor_reduce(
                            msum[:],
                            of[:].rearrange("p (h c) -> p c h", h=H),
                            axis=mybir.AxisListType.X, op=OP.add)
                        nc.sync.dma_start(out_d[w * P:w * P + nw, :], msum[0:nw, :])

            for _rep in range(repeat):
                for l in range(3):
                    dense_phase(l)
                    nc.gpsimd.collective_compute(
                        "AllGather", OP.bypass,
                        replica_groups=[list(range(NCORES))],
                        ins=[tin_rows[l][:, :].opt()],
                        outs=[tab[l][:, :].opt()])
                    edge_phase(l)

    nc.compile()
    return nc


# ======================= entry point =======================

def kernel(**inputs) -> np.ndarray:
    from concourse.bass_utils import run_bass_kernel_spmd

    in_maps, meta_prog = _preprocess(inputs)
    key = (meta_prog["KA"], meta_prog["KB"], meta_prog["use_bias"])
    if key not in _CACHE:
        _CACHE[key] = _build(meta_prog)
    nc = _CACHE[key]
    res = run_bass_kernel_spmd(nc, in_maps, core_ids=list(range(NCORES)))
    return np.concatenate([r["out"] for r in res.results], axis=0)
